# revision 9
# baseline (speedup 1.0000x reference)
"""Cross-attention kernel for Trainium2 (8 NeuronCores, data-parallel over batch).

Computation (per batch element b, H=16 heads, D=64 head dim, C=1024):
    Q  = x_b @ q_w                      [1024, 1024]
    K  = context @ kv_w[:, :1024]       [2048, 1024]
    V  = context @ kv_w[:, 1024:]       [2048, 1024]
    S_h = (Q_h K_h^T) / sqrt(D)         [1024, 2048] per head
    P_h = softmax(S_h, axis=-1)
    O_h = P_h V_h                       [1024, 64]
    out = concat_h(O_h) @ proj_w + proj_b

Layout strategy (everything flows without intermediate transposes except the
initial x / context transposes):
  - x_T [c, q], ctx_T [c, kv] built with PE transposes.
  - QT [hd, q] and KT [hd, kv] computed "transposed" (weights stationary).
  - V [kv, hd] computed "natural" (ctx_T stationary).
  - S_T[k, q] = KT_h^T-slices stationary, QT_h moving (contraction d=64).
  - exp fused on ACT (scale=1/8 folded in); no max subtraction (scores ~N(0,1)).
  - P@V: V'_h=[V_h | 1] stationary => O^T[d, q] plus denominator s[q] in
    partition 64 of the same PSUM accumulation.
  - final proj: O^T slices stationary, proj_w moving => out in natural [q, c].
"""

import sys

if "/opt/trn_rl_repo" not in sys.path:
    sys.path.insert(0, "/opt/trn_rl_repo")

import numpy as np

import concourse.bass as bass
import concourse.tile as tile
from concourse import bacc, mybir
from concourse.bass_utils import run_bass_kernel_spmd
from concourse.masks import make_identity

F32 = mybir.dt.float32

B = 8
NQ = 1024
NKV = 2048
C = 1024
H = 16
D = 64
P = 128
SCALE = D ** -0.5

# Matmul input dtype: float32 (exact) or float32r (fast, reduced precision).
MM_DT = mybir.dt.float32


def _mm_ap(ap):
    if MM_DT is F32:
        return ap
    return ap.bitcast(MM_DT)


def _build_kernel():
    nc = bacc.Bacc("TRN2", target_bir_lowering=False, debug=False)

    x_in = nc.dram_tensor("x", [NQ, C], F32, kind="ExternalInput").ap()
    ctx_in = nc.dram_tensor("context", [NKV, C], F32, kind="ExternalInput").ap()
    qw_in = nc.dram_tensor("q_w", [C, C], F32, kind="ExternalInput").ap()
    kvw_in = nc.dram_tensor("kv_w", [C, 2 * C], F32, kind="ExternalInput").ap()
    pw_in = nc.dram_tensor("proj_w", [C, C], F32, kind="ExternalInput").ap()
    pb_in = nc.dram_tensor("proj_b", [C], F32, kind="ExternalInput").ap()
    out_d = nc.dram_tensor("out", [NQ, C], F32, kind="ExternalOutput").ap()

    with tile.TileContext(nc) as tc:
        _emit(nc, tc, x_in, ctx_in, qw_in, kvw_in, pw_in, pb_in, out_d)

    nc.compile()
    return nc


def _emit(nc, tc, x_in, ctx_in, qw_in, kvw_in, pw_in, pb_in, out_d):
    from contextlib import ExitStack

    ctx = ExitStack()
    with ctx:
        dram = ctx.enter_context(tc.tile_pool(name="dram", bufs=1, space="DRAM"))
        # KT in DRAM: plane p holds heads 2p (rows 0:64) and 2p+1 (rows 64:128)
        kt_dram = dram.tile([H // 2, P, NKV], F32)
        # V natural: [kv_tile, 128, hd]
        v_dram = dram.tile([NKV // P, P, C], F32)
        rdram = ctx.enter_context(tc.tile_pool(name="rdram", bufs=4, space="DRAM"))

        persist = ctx.enter_context(tc.tile_pool(name="persist", bufs=1))
        qt_sb = persist.tile([P, C // P, NQ], F32)      # QT [hd, q]: 32KB/p
        ident = persist.tile([P, P], F32)
        make_identity(nc, ident)

        # ---------------- Phase A+B: x -> x_T -> QT ----------------
        with tc.tile_pool(name="xab", bufs=1) as xab, \
             tc.tile_pool(name="ldA", bufs=3) as ldA, \
             tc.tile_pool(name="wq", bufs=8) as wqp, \
             tc.tile_pool(name="pst", bufs=2, space="PSUM") as pst, \
             tc.tile_pool(name="psp", bufs=4, space="PSUM") as psp:
            x_t = xab.tile([P, C // P, NQ], F32)        # x^T [c, q]: 32KB/p
            for qt in range(NQ // P):
                xa = ldA.tile([P, C], F32, tag="xa")
                nc.sync.dma_start(xa, x_in[qt * P:(qt + 1) * P, :])
                for ct in range(C // P):
                    ps = pst.tile([P, P], F32)
                    nc.tensor.transpose(ps, xa[:, ct * P:(ct + 1) * P], ident)
                    nc.vector.tensor_copy(x_t[:, ct, qt * P:(qt + 1) * P], ps)

            qw_sb = []
            for c in range(C // P):
                w = wqp.tile([P, C], F32, tag="qw")
                nc.sync.dma_start(w, qw_in[c * P:(c + 1) * P, :])
                qw_sb.append(w)
            for mt in range(C // P):
                for qch in range(NQ // 512):
                    ps = psp.tile([P, 512], F32)
                    for c in range(C // P):
                        nc.tensor.matmul(
                            ps,
                            _mm_ap(qw_sb[c][:, mt * P:(mt + 1) * P]),
                            _mm_ap(x_t[:, c, qch * 512:(qch + 1) * 512]),
                            start=(c == 0), stop=(c == C // P - 1))
                    nc.vector.tensor_copy(qt_sb[:, mt, qch * 512:(qch + 1) * 512], ps)

        # ---------------- Phase C+D: context -> ctx_T -> KT, V ----------------
        with tc.tile_pool(name="cd", bufs=1) as cd, \
             tc.tile_pool(name="ldC", bufs=3) as ldC, \
             tc.tile_pool(name="wkv", bufs=8) as wkv, \
             tc.tile_pool(name="ev", bufs=4) as ev, \
             tc.tile_pool(name="pst", bufs=2, space="PSUM") as pst, \
             tc.tile_pool(name="psp", bufs=4, space="PSUM") as psp:
            ctx_t = cd.tile([P, C // P, NKV], F32)      # ctx^T [c, kv]: 64KB/p
            for kt in range(NKV // P):
                ca = ldC.tile([P, C], F32, tag="ca")
                nc.sync.dma_start(ca, ctx_in[kt * P:(kt + 1) * P, :])
                for ct in range(C // P):
                    ps = pst.tile([P, P], F32)
                    nc.tensor.transpose(ps, ca[:, ct * P:(ct + 1) * P], ident)
                    nc.vector.tensor_copy(ctx_t[:, ct, kt * P:(kt + 1) * P], ps)

            kvw_k = []
            for c in range(C // P):
                wk = wkv.tile([P, C], F32, tag="wkv")
                nc.sync.dma_start(wk, kvw_in[c * P:(c + 1) * P, 0:C])
                kvw_k.append(wk)

            # KT [k_col_tile, kv]: kvw_k stationary, ctx_T moving
            for colt in range(C // P):
                for kvch in range(NKV // 512):
                    ps = psp.tile([P, 512], F32)
                    for c in range(C // P):
                        nc.tensor.matmul(
                            ps,
                            _mm_ap(kvw_k[c][:, colt * P:(colt + 1) * P]),
                            _mm_ap(ctx_t[:, c, kvch * 512:(kvch + 1) * 512]),
                            start=(c == 0), stop=(c == C // P - 1))
                    st = ev.tile([P, 512], F32, tag="kst")
                    nc.scalar.copy(st, ps)
                    nc.sync.dma_start(
                        kt_dram[colt, :, kvch * 512:(kvch + 1) * 512], st)

            # V [kv_tile, hd]: ctx_T stationary, kvw_v moving.
            # kvw_v tiles reuse the wkv slots (WAR deps handled by Tile).
            kvw_v = []
            for c in range(C // P):
                wv = wkv.tile([P, C], F32, tag="wkv")
                nc.sync.dma_start(wv, kvw_in[c * P:(c + 1) * P, C:2 * C])
                kvw_v.append(wv)
            for vt in range(NKV // P):
                for hdch in range(C // 512):
                    ps = psp.tile([P, 512], F32)
                    for c in range(C // P):
                        nc.tensor.matmul(
                            ps,
                            _mm_ap(ctx_t[:, c, vt * P:(vt + 1) * P]),
                            _mm_ap(kvw_v[c][:, hdch * 512:(hdch + 1) * 512]),
                            start=(c == 0), stop=(c == C // P - 1))
                    st = ev.tile([P, 512], F32, tag="vst")
                    nc.vector.tensor_copy(st, ps)
                    nc.sync.dma_start(
                        v_dram[vt, :, hdch * 512:(hdch + 1) * 512], st)

        # ---------------- Phase E: attention per head pair ----------------
        o_pool = ctx.enter_context(tc.tile_pool(name="o_pool", bufs=1))
        o_sb = o_pool.tile([P, C // P, NQ], F32)        # O^T [hd, q]: 32KB/p

        NKT = NKV // P  # 16 k tiles
        with tc.tile_pool(name="kv_e", bufs=2) as kv_e, \
             tc.tile_pool(name="epool", bufs=2 * NKT + 2) as epool, \
             tc.tile_pool(name="rp", bufs=4) as rp, \
             tc.tile_pool(name="ost", bufs=2) as ostp, \
             tc.tile_pool(name="ps_s", bufs=4, space="PSUM") as ps_s, \
             tc.tile_pool(name="ps_pv", bufs=2, space="PSUM") as ps_pv:
            for hp in range(H // 2):
                ktp = kv_e.tile([P, NKV], F32, tag="ktp")     # 8KB/p
                nc.sync.dma_start(ktp, kt_dram[hp])
                vp = kv_e.tile([P, NKT, 2, D + 1], F32, tag="vp")  # 8.1KB/p
                nc.vector.memset(vp[:, :, :, D:D + 1], 1.0)
                for hh in range(2):
                    h = 2 * hp + hh
                    nc.sync.dma_start(
                        vp[:, :, hh, 0:D],
                        v_dram[:, :, h * D:(h + 1) * D].transpose([1, 0, 2]))

                for qh in range(NQ // 512):
                    qs = slice(qh * 512, (qh + 1) * 512)
                    e_tiles = [[None] * NKT for _ in range(2)]
                    for kt in range(NKT):
                        for hh in range(2):
                            ps = ps_s.tile([P, 512], F32)
                            nc.tensor.matmul(
                                ps,
                                _mm_ap(ktp[hh * D:(hh + 1) * D,
                                           kt * P:(kt + 1) * P]),
                                _mm_ap(qt_sb[hh * D:(hh + 1) * D, hp, qs]),
                                start=True, stop=True)
                            et = epool.tile([P, 512], F32, tag="e")
                            nc.scalar.activation(
                                et, ps, mybir.ActivationFunctionType.Exp,
                                scale=SCALE)
                            e_tiles[hh][kt] = et
                    for hh in range(2):
                        h = 2 * hp + hh
                        pso = ps_pv.tile([P, 512], F32)
                        for kt in range(NKT):
                            nc.tensor.matmul(
                                pso[0:D + 1, :],
                                _mm_ap(vp[:, kt, hh, :]),
                                _mm_ap(e_tiles[hh][kt]),
                                start=(kt == 0), stop=(kt == NKT - 1))
                        # reciprocal of the softmax denominator (row 64),
                        # broadcast to 64 partitions via a DRAM bounce
                        # (SBUF-source partition-step-0 DMA is illegal).
                        rrow = rp.tile([P, 512], F32, tag="rrow")
                        nc.vector.reciprocal(rrow[D:D + 1, :], pso[D:D + 1, :])
                        rd = rdram.tile([1, 512], F32, tag="rd")
                        nc.sync.dma_start(rd, rrow[D:D + 1, :])
                        rbc = rp.tile([D, 512], F32, tag="rbc")
                        nc.sync.dma_start(
                            rbc, rd.partition_broadcast(D))
                        if hh == 0:
                            nc.vector.tensor_mul(
                                o_sb[0:D, hp, qs], pso[0:D, :], rbc)
                        else:
                            ost = ostp.tile([D, 512], F32, tag="ost")
                            nc.vector.tensor_mul(ost, pso[0:D, :], rbc)
                            nc.sync.dma_start(o_sb[D:2 * D, hp, qs], ost)

        # ---------------- Phase F: final projection ----------------
        with tc.tile_pool(name="wp", bufs=9) as wpp, \
             tc.tile_pool(name="fin", bufs=3) as finp, \
             tc.tile_pool(name="psp", bufs=4, space="PSUM") as psp:
            bias_bc = wpp.tile([P, C], F32, tag="bias")
            pb2 = pb_in.unsqueeze(0)  # [1, C]
            nc.sync.dma_start(bias_bc, pb2.partition_broadcast(P))
            pw_sb = []
            for hc in range(C // P):
                w = wpp.tile([P, C], F32, tag="pw")
                nc.sync.dma_start(w, pw_in[hc * P:(hc + 1) * P, :])
                pw_sb.append(w)
            for qt in range(NQ // P):
                for cch in range(C // 512):
                    ps = psp.tile([P, 512], F32)
                    for hc in range(C // P):
                        nc.tensor.matmul(
                            ps,
                            _mm_ap(o_sb[:, hc, qt * P:(qt + 1) * P]),
                            _mm_ap(pw_sb[hc][:, cch * 512:(cch + 1) * 512]),
                            start=(hc == 0), stop=(hc == C // P - 1))
                    ft = finp.tile([P, 512], F32, tag="fin")
                    nc.vector.tensor_add(ft, ps, bias_bc[:, cch * 512:(cch + 1) * 512])
                    nc.sync.dma_start(
                        out_d[qt * P:(qt + 1) * P, cch * 512:(cch + 1) * 512], ft)


_CACHED_NC = None


def _get_nc():
    global _CACHED_NC
    if _CACHED_NC is None:
        _CACHED_NC = _build_kernel()
    return _CACHED_NC


def kernel(x, context, q_w, kv_w, proj_w, proj_b):
    x = np.ascontiguousarray(np.asarray(x, dtype=np.float32))
    context = np.ascontiguousarray(np.asarray(context, dtype=np.float32))
    q_w = np.ascontiguousarray(np.asarray(q_w, dtype=np.float32))
    kv_w = np.ascontiguousarray(np.asarray(kv_w, dtype=np.float32))
    proj_w = np.ascontiguousarray(np.asarray(proj_w, dtype=np.float32))
    proj_b = np.ascontiguousarray(np.asarray(proj_b, dtype=np.float32))

    nc = _get_nc()
    in_maps = [
        {
            "x": x[i],
            "context": context,
            "q_w": q_w,
            "kv_w": kv_w,
            "proj_w": proj_w,
            "proj_b": proj_b,
        }
        for i in range(B)
    ]
    res = run_bass_kernel_spmd(nc, in_maps, core_ids=list(range(B)))
    out = np.stack([res.results[i]["out"] for i in range(B)], axis=0)
    return out


# revision 13
# speedup vs baseline: 1.0190x; 1.0190x over previous
"""Cross-attention kernel for Trainium2 (8 NeuronCores, data-parallel over batch).

Computation (per batch element b, H=16 heads, D=64 head dim, C=1024):
    Q  = x_b @ q_w                      [1024, 1024]
    K  = context @ kv_w[:, :1024]       [2048, 1024]
    V  = context @ kv_w[:, 1024:]       [2048, 1024]
    S_h = (Q_h K_h^T) / sqrt(D)         [1024, 2048] per head
    P_h = softmax(S_h, axis=-1)
    O_h = P_h V_h                       [1024, 64]
    out = concat_h(O_h) @ proj_w + proj_b

Layout strategy (everything flows without intermediate transposes except the
initial x / context transposes):
  - x_T [c, q], ctx_T [c, kv] built with PE transposes.
  - QT [hd, q] and KT [hd, kv] computed "transposed" (weights stationary).
  - V [kv, hd] computed "natural" (ctx_T stationary).
  - S_T[k, q] = KT_h^T-slices stationary, QT_h moving (contraction d=64).
  - exp fused on ACT (scale=1/8 folded in); no max subtraction (scores ~N(0,1)).
  - P@V: V'_h=[V_h | 1] stationary => O^T[d, q] plus denominator s[q] in
    partition 64 of the same PSUM accumulation.
  - final proj: O^T slices stationary, proj_w moving => out in natural [q, c].
"""

import sys

if "/opt/trn_rl_repo" not in sys.path:
    sys.path.insert(0, "/opt/trn_rl_repo")

import numpy as np

import concourse.bass as bass
import concourse.tile as tile
from concourse import bacc, mybir
from concourse.bass_utils import run_bass_kernel_spmd
from concourse.masks import make_identity

F32 = mybir.dt.float32

B = 8
NQ = 1024
NKV = 2048
C = 1024
H = 16
D = 64
P = 128
SCALE = D ** -0.5

# Matmul input dtype: float32 (exact) or float32r (fast, reduced precision).
MM_DT = mybir.dt.float32

# Debug: stop after N phases (1=A+B, 2=+C/D, 3=+E, 4=all). For bisection only.
import os
PHASES = int(os.environ.get("K_PHASES", "4"))


def _mm_ap(ap):
    if MM_DT is F32:
        return ap
    return ap.bitcast(MM_DT)


def _build_kernel():
    nc = bacc.Bacc("TRN2", target_bir_lowering=False, debug=False)

    x_in = nc.dram_tensor("x", [NQ, C], F32, kind="ExternalInput").ap()
    ctx_in = nc.dram_tensor("context", [NKV, C], F32, kind="ExternalInput").ap()
    qw_in = nc.dram_tensor("q_w", [C, C], F32, kind="ExternalInput").ap()
    kvw_in = nc.dram_tensor("kv_w", [C, 2 * C], F32, kind="ExternalInput").ap()
    pw_in = nc.dram_tensor("proj_w", [C, C], F32, kind="ExternalInput").ap()
    pb_in = nc.dram_tensor("proj_b", [C], F32, kind="ExternalInput").ap()
    out_d = nc.dram_tensor("out", [NQ, C], F32, kind="ExternalOutput").ap()

    with tile.TileContext(nc) as tc:
        _emit(nc, tc, x_in, ctx_in, qw_in, kvw_in, pw_in, pb_in, out_d)

    nc.compile()
    return nc


def _emit(nc, tc, x_in, ctx_in, qw_in, kvw_in, pw_in, pb_in, out_d):
    from contextlib import ExitStack

    ctx = ExitStack()
    with ctx:
        dram = ctx.enter_context(tc.tile_pool(name="dram", bufs=1, space="DRAM"))
        # KT in DRAM: plane p holds heads 2p (rows 0:64) and 2p+1 (rows 64:128)
        kt_dram = dram.tile([H // 2, P, NKV], F32)
        # V natural: [kv_tile, 128, hd]
        v_dram = dram.tile([NKV // P, P, C], F32)
        rdram = ctx.enter_context(tc.tile_pool(name="rdram", bufs=4, space="DRAM"))

        persist = ctx.enter_context(tc.tile_pool(name="persist", bufs=1))
        qt_sb = persist.tile([P, C // P, NQ], F32)      # QT [hd, q]: 32KB/p
        ident = persist.tile([P, P], F32)
        make_identity(nc, ident)

        # ---------------- Phase A+B: x -> x_T -> QT ----------------
        with tc.tile_pool(name="xab", bufs=1) as xab, \
             tc.tile_pool(name="ldA", bufs=3) as ldA, \
             tc.tile_pool(name="wq", bufs=8) as wqp, \
             tc.tile_pool(name="pst", bufs=2, space="PSUM") as pst, \
             tc.tile_pool(name="psp", bufs=4, space="PSUM") as psp:
            x_t = xab.tile([P, C // P, NQ], F32)        # x^T [c, q]: 32KB/p
            for qt in range(NQ // P):
                xa = ldA.tile([P, C], F32, tag="xa")
                nc.sync.dma_start(xa, x_in[qt * P:(qt + 1) * P, :])
                for ct in range(C // P):
                    ps = pst.tile([P, P], F32)
                    nc.tensor.transpose(ps, xa[:, ct * P:(ct + 1) * P], ident)
                    nc.vector.tensor_copy(x_t[:, ct, qt * P:(qt + 1) * P], ps)

            qw_sb = []
            for c in range(C // P):
                w = wqp.tile([P, C], F32, tag="qw")
                nc.sync.dma_start(w, qw_in[c * P:(c + 1) * P, :])
                qw_sb.append(w)
            for mt in range(C // P):
                for qch in range(NQ // 512):
                    ps = psp.tile([P, 512], F32)
                    for c in range(C // P):
                        nc.tensor.matmul(
                            ps,
                            _mm_ap(qw_sb[c][:, mt * P:(mt + 1) * P]),
                            _mm_ap(x_t[:, c, qch * 512:(qch + 1) * 512]),
                            start=(c == 0), stop=(c == C // P - 1))
                    nc.vector.tensor_copy(qt_sb[:, mt, qch * 512:(qch + 1) * 512], ps)

        # ---------------- Phase C+D: context -> ctx_T -> KT, V ----------------
        if PHASES < 2:
            return
        with tc.tile_pool(name="cd", bufs=1) as cd, \
             tc.tile_pool(name="ldC", bufs=3) as ldC, \
             tc.tile_pool(name="wkv", bufs=8) as wkv, \
             tc.tile_pool(name="ev", bufs=4) as ev, \
             tc.tile_pool(name="pst", bufs=2, space="PSUM") as pst, \
             tc.tile_pool(name="psp", bufs=4, space="PSUM") as psp:
            ctx_t = cd.tile([P, C // P, NKV], F32)      # ctx^T [c, kv]: 64KB/p
            for kt in range(NKV // P):
                ca = ldC.tile([P, C], F32, tag="ca")
                nc.sync.dma_start(ca, ctx_in[kt * P:(kt + 1) * P, :])
                for ct in range(C // P):
                    ps = pst.tile([P, P], F32)
                    nc.tensor.transpose(ps, ca[:, ct * P:(ct + 1) * P], ident)
                    nc.vector.tensor_copy(ctx_t[:, ct, kt * P:(kt + 1) * P], ps)

            kvw_k = []
            for c in range(C // P):
                wk = wkv.tile([P, C], F32, tag="wkv")
                nc.sync.dma_start(wk, kvw_in[c * P:(c + 1) * P, 0:C])
                kvw_k.append(wk)

            # KT [k_col_tile, kv]: kvw_k stationary, ctx_T moving
            for colt in range(C // P):
                for kvch in range(NKV // 512):
                    ps = psp.tile([P, 512], F32)
                    for c in range(C // P):
                        nc.tensor.matmul(
                            ps,
                            _mm_ap(kvw_k[c][:, colt * P:(colt + 1) * P]),
                            _mm_ap(ctx_t[:, c, kvch * 512:(kvch + 1) * 512]),
                            start=(c == 0), stop=(c == C // P - 1))
                    st = ev.tile([P, 512], F32, tag="kst")
                    nc.scalar.copy(st, ps)
                    nc.sync.dma_start(
                        kt_dram[colt, :, kvch * 512:(kvch + 1) * 512], st)

            # V [kv_tile, hd]: ctx_T stationary, kvw_v moving.
            # kvw_v tiles reuse the wkv slots (WAR deps handled by Tile).
            kvw_v = []
            for c in range(C // P):
                wv = wkv.tile([P, C], F32, tag="wkv")
                nc.sync.dma_start(wv, kvw_in[c * P:(c + 1) * P, C:2 * C])
                kvw_v.append(wv)
            for vt in range(NKV // P):
                for hdch in range(C // 512):
                    ps = psp.tile([P, 512], F32)
                    for c in range(C // P):
                        nc.tensor.matmul(
                            ps,
                            _mm_ap(ctx_t[:, c, vt * P:(vt + 1) * P]),
                            _mm_ap(kvw_v[c][:, hdch * 512:(hdch + 1) * 512]),
                            start=(c == 0), stop=(c == C // P - 1))
                    st = ev.tile([P, 512], F32, tag="vst")
                    nc.vector.tensor_copy(st, ps)
                    nc.sync.dma_start(
                        v_dram[vt, :, hdch * 512:(hdch + 1) * 512], st)

        # ---------------- Phase E: attention per head pair ----------------
        if PHASES < 3:
            return
        o_pool = ctx.enter_context(tc.tile_pool(name="o_pool", bufs=1))
        o_sb = o_pool.tile([P, C // P, NQ], F32)        # O^T [hd, q]: 32KB/p

        NKT = NKV // P  # 16 k tiles
        with tc.tile_pool(name="kv_e", bufs=2) as kv_e, \
             tc.tile_pool(name="epool", bufs=2 * NKT + 2) as epool, \
             tc.tile_pool(name="rp", bufs=4) as rp, \
             tc.tile_pool(name="ost", bufs=2) as ostp, \
             tc.tile_pool(name="ps_s", bufs=4, space="PSUM") as ps_s, \
             tc.tile_pool(name="ps_pv", bufs=2, space="PSUM") as ps_pv:
            for hp in range(H // 2):
                ktp = kv_e.tile([P, NKV], F32, tag="ktp")     # 8KB/p
                nc.sync.dma_start(ktp, kt_dram[hp])
                vp = kv_e.tile([P, NKT, 2, D + 1], F32, tag="vp")  # 8.1KB/p
                nc.vector.memset(vp[:, :, :, D:D + 1], 1.0)
                for hh in range(2):
                    h = 2 * hp + hh
                    nc.sync.dma_start(
                        vp[:, :, hh, 0:D],
                        v_dram[:, :, h * D:(h + 1) * D].transpose([1, 0, 2]))

                for qh in range(NQ // 512):
                    qs = slice(qh * 512, (qh + 1) * 512)
                    e_tiles = [[None] * NKT for _ in range(2)]
                    for kt in range(NKT):
                        for hh in range(2):
                            ps = ps_s.tile([P, 512], F32)
                            nc.tensor.matmul(
                                ps,
                                _mm_ap(ktp[hh * D:(hh + 1) * D,
                                           kt * P:(kt + 1) * P]),
                                _mm_ap(qt_sb[hh * D:(hh + 1) * D, hp, qs]),
                                start=True, stop=True)
                            et = epool.tile([P, 512], F32, tag="e")
                            nc.scalar.activation(
                                et, ps, mybir.ActivationFunctionType.Exp,
                                scale=SCALE)
                            e_tiles[hh][kt] = et
                    for hh in range(2):
                        h = 2 * hp + hh
                        pso = ps_pv.tile([P, 512], F32)
                        for kt in range(NKT):
                            nc.tensor.matmul(
                                pso[0:D + 1, :],
                                _mm_ap(vp[:, kt, hh, :]),
                                _mm_ap(e_tiles[hh][kt]),
                                start=(kt == 0), stop=(kt == NKT - 1))
                        # reciprocal of the softmax denominator (row 64),
                        # broadcast to 64 partitions via a DRAM bounce
                        # (SBUF-source partition-step-0 DMA is illegal).
                        rrow = rp.tile([P, 512], F32, tag="rrow")
                        nc.vector.reciprocal(rrow[D:D + 1, :], pso[D:D + 1, :])
                        rd = rdram.tile([1, 512], F32, tag="rd")
                        nc.sync.dma_start(rd, rrow[D:D + 1, :])
                        rbc = rp.tile([D, 512], F32, tag="rbc")
                        nc.sync.dma_start(
                            rbc, rd.partition_broadcast(D))
                        if hh == 0:
                            nc.vector.tensor_mul(
                                o_sb[0:D, hp, qs], pso[0:D, :], rbc)
                        else:
                            ost = ostp.tile([D, 512], F32, tag="ost")
                            nc.vector.tensor_mul(ost, pso[0:D, :], rbc)
                            nc.sync.dma_start(o_sb[D:2 * D, hp, qs], ost)

        # ---------------- Phase F: final projection ----------------
        if PHASES < 4:
            return
        with tc.tile_pool(name="wp", bufs=9) as wpp, \
             tc.tile_pool(name="fin", bufs=3) as finp, \
             tc.tile_pool(name="psp", bufs=4, space="PSUM") as psp:
            bias_bc = wpp.tile([P, C], F32, tag="bias")
            pb2 = pb_in.unsqueeze(0)  # [1, C]
            nc.sync.dma_start(bias_bc, pb2.partition_broadcast(P))
            pw_sb = []
            for hc in range(C // P):
                w = wpp.tile([P, C], F32, tag="pw")
                nc.sync.dma_start(w, pw_in[hc * P:(hc + 1) * P, :])
                pw_sb.append(w)
            for qt in range(NQ // P):
                for cch in range(C // 512):
                    ps = psp.tile([P, 512], F32)
                    for hc in range(C // P):
                        nc.tensor.matmul(
                            ps,
                            _mm_ap(o_sb[:, hc, qt * P:(qt + 1) * P]),
                            _mm_ap(pw_sb[hc][:, cch * 512:(cch + 1) * 512]),
                            start=(hc == 0), stop=(hc == C // P - 1))
                    ft = finp.tile([P, 512], F32, tag="fin")
                    nc.vector.tensor_add(ft, ps, bias_bc[:, cch * 512:(cch + 1) * 512])
                    nc.sync.dma_start(
                        out_d[qt * P:(qt + 1) * P, cch * 512:(cch + 1) * 512], ft)


_CACHED_NC = None


def _get_nc():
    global _CACHED_NC
    if _CACHED_NC is None:
        _CACHED_NC = _build_kernel()
    return _CACHED_NC


def kernel(x, context, q_w, kv_w, proj_w, proj_b):
    x = np.ascontiguousarray(np.asarray(x, dtype=np.float32))
    context = np.ascontiguousarray(np.asarray(context, dtype=np.float32))
    q_w = np.ascontiguousarray(np.asarray(q_w, dtype=np.float32))
    kv_w = np.ascontiguousarray(np.asarray(kv_w, dtype=np.float32))
    proj_w = np.ascontiguousarray(np.asarray(proj_w, dtype=np.float32))
    proj_b = np.ascontiguousarray(np.asarray(proj_b, dtype=np.float32))

    nc = _get_nc()
    in_maps = [
        {
            "x": x[i],
            "context": context,
            "q_w": q_w,
            "kv_w": kv_w,
            "proj_w": proj_w,
            "proj_b": proj_b,
        }
        for i in range(B)
    ]
    res = run_bass_kernel_spmd(nc, in_maps, core_ids=list(range(B)))
    out = np.stack([res.results[i]["out"] for i in range(B)], axis=0)
    return out


# revision 14
# speedup vs baseline: 31.0074x; 30.4304x over previous
"""Cross-attention kernel for Trainium2 (8 NeuronCores, data-parallel over batch).

Computation (per batch element b, H=16 heads, D=64 head dim, C=1024):
    Q  = x_b @ q_w                      [1024, 1024]
    K  = context @ kv_w[:, :1024]       [2048, 1024]
    V  = context @ kv_w[:, 1024:]       [2048, 1024]
    S_h = (Q_h K_h^T) / sqrt(D)         [1024, 2048] per head
    P_h = softmax(S_h, axis=-1)
    O_h = P_h V_h                       [1024, 64]
    out = concat_h(O_h) @ proj_w + proj_b

Layout strategy (everything flows without intermediate transposes except the
initial x / context transposes):
  - x_T [c, q], ctx_T [c, kv] built with PE transposes.
  - QT [hd, q] and KT [hd, kv] computed "transposed" (weights stationary).
  - V [kv, hd] computed "natural" (ctx_T stationary).
  - S_T[k, q] = KT_h^T-slices stationary, QT_h moving (contraction d=64).
  - exp fused on ACT (scale=1/8 folded in); no max subtraction (scores ~N(0,1)).
  - P@V: V'_h=[V_h | 1] stationary => O^T[d, q] plus denominator s[q] in
    partition 64 of the same PSUM accumulation.
  - final proj: O^T slices stationary, proj_w moving => out in natural [q, c].
"""

import sys

if "/opt/trn_rl_repo" not in sys.path:
    sys.path.insert(0, "/opt/trn_rl_repo")

import numpy as np

import concourse.bass as bass
import concourse.tile as tile
from concourse import bacc, mybir
from concourse.bass_utils import run_bass_kernel_spmd
from concourse.masks import make_identity

F32 = mybir.dt.float32

B = 8
NQ = 1024
NKV = 2048
C = 1024
H = 16
D = 64
P = 128
SCALE = D ** -0.5

# Matmul input dtype: float32 (exact) or float32r (fast, reduced precision).
MM_DT = mybir.dt.float32

# Debug: stop after N phases (1=A+B, 2=+C/D, 3=+E, 4=all). For bisection only.
import os
PHASES = int(os.environ.get("K_PHASES", "4"))
REPEAT = int(os.environ.get("K_REPEAT", "1"))


def _mm_ap(ap):
    if MM_DT is F32:
        return ap
    return ap.bitcast(MM_DT)


def _build_kernel():
    nc = bacc.Bacc("TRN2", target_bir_lowering=False, debug=False)

    x_in = nc.dram_tensor("x", [NQ, C], F32, kind="ExternalInput").ap()
    ctx_in = nc.dram_tensor("context", [NKV, C], F32, kind="ExternalInput").ap()
    qw_in = nc.dram_tensor("q_w", [C, C], F32, kind="ExternalInput").ap()
    kvw_in = nc.dram_tensor("kv_w", [C, 2 * C], F32, kind="ExternalInput").ap()
    pw_in = nc.dram_tensor("proj_w", [C, C], F32, kind="ExternalInput").ap()
    pb_in = nc.dram_tensor("proj_b", [C], F32, kind="ExternalInput").ap()
    out_d = nc.dram_tensor("out", [NQ, C], F32, kind="ExternalOutput").ap()

    with tile.TileContext(nc) as tc:
        _emit(nc, tc, x_in, ctx_in, qw_in, kvw_in, pw_in, pb_in, out_d)

    nc.compile()
    return nc


def _emit(nc, tc, x_in, ctx_in, qw_in, kvw_in, pw_in, pb_in, out_d):
    from contextlib import ExitStack

    ctx = ExitStack()
    with ctx:
        dram = ctx.enter_context(tc.tile_pool(name="dram", bufs=1, space="DRAM"))
        # KT in DRAM: plane p holds heads 2p (rows 0:64) and 2p+1 (rows 64:128)
        kt_dram = dram.tile([H // 2, P, NKV], F32)
        # V natural: [kv_tile, 128, hd]
        v_dram = dram.tile([NKV // P, P, C], F32)
        rdram = ctx.enter_context(tc.tile_pool(name="rdram", bufs=4, space="DRAM"))

        for _rep in range(REPEAT):
            _emit_body(nc, tc, _rep, kt_dram, v_dram, rdram,
                       x_in, ctx_in, qw_in, kvw_in, pw_in, pb_in, out_d)


def _emit_body(nc, tc, rep, kt_dram, v_dram, rdram,
               x_in, ctx_in, qw_in, kvw_in, pw_in, pb_in, out_d):
    from contextlib import ExitStack
    ctx = ExitStack()
    with ctx:
        persist = ctx.enter_context(tc.tile_pool(name=f"persist{rep}", bufs=1))
        qt_sb = persist.tile([P, C // P, NQ], F32)      # QT [hd, q]: 32KB/p
        ident = persist.tile([P, P], F32)
        make_identity(nc, ident)

        # ---------------- Phase A+B: x -> x_T -> QT ----------------
        with tc.tile_pool(name="xab", bufs=1) as xab, \
             tc.tile_pool(name="ldA", bufs=3) as ldA, \
             tc.tile_pool(name="wq", bufs=8) as wqp, \
             tc.tile_pool(name="pst", bufs=2, space="PSUM") as pst, \
             tc.tile_pool(name="psp", bufs=4, space="PSUM") as psp:
            x_t = xab.tile([P, C // P, NQ], F32)        # x^T [c, q]: 32KB/p
            for qt in range(NQ // P):
                xa = ldA.tile([P, C], F32, tag="xa")
                nc.sync.dma_start(xa, x_in[qt * P:(qt + 1) * P, :])
                for ct in range(C // P):
                    ps = pst.tile([P, P], F32)
                    nc.tensor.transpose(ps, xa[:, ct * P:(ct + 1) * P], ident)
                    nc.vector.tensor_copy(x_t[:, ct, qt * P:(qt + 1) * P], ps)

            qw_sb = []
            for c in range(C // P):
                w = wqp.tile([P, C], F32, tag="qw")
                nc.sync.dma_start(w, qw_in[c * P:(c + 1) * P, :])
                qw_sb.append(w)
            for mt in range(C // P):
                for qch in range(NQ // 512):
                    ps = psp.tile([P, 512], F32)
                    for c in range(C // P):
                        nc.tensor.matmul(
                            ps,
                            _mm_ap(qw_sb[c][:, mt * P:(mt + 1) * P]),
                            _mm_ap(x_t[:, c, qch * 512:(qch + 1) * 512]),
                            start=(c == 0), stop=(c == C // P - 1))
                    nc.vector.tensor_copy(qt_sb[:, mt, qch * 512:(qch + 1) * 512], ps)

        # ---------------- Phase C+D: context -> ctx_T -> KT, V ----------------
        if PHASES < 2:
            return
        with tc.tile_pool(name="cd", bufs=1) as cd, \
             tc.tile_pool(name="ldC", bufs=3) as ldC, \
             tc.tile_pool(name="wkv", bufs=8) as wkv, \
             tc.tile_pool(name="ev", bufs=4) as ev, \
             tc.tile_pool(name="pst", bufs=2, space="PSUM") as pst, \
             tc.tile_pool(name="psp", bufs=4, space="PSUM") as psp:
            ctx_t = cd.tile([P, C // P, NKV], F32)      # ctx^T [c, kv]: 64KB/p
            for kt in range(NKV // P):
                ca = ldC.tile([P, C], F32, tag="ca")
                nc.sync.dma_start(ca, ctx_in[kt * P:(kt + 1) * P, :])
                for ct in range(C // P):
                    ps = pst.tile([P, P], F32)
                    nc.tensor.transpose(ps, ca[:, ct * P:(ct + 1) * P], ident)
                    nc.vector.tensor_copy(ctx_t[:, ct, kt * P:(kt + 1) * P], ps)

            kvw_k = []
            for c in range(C // P):
                wk = wkv.tile([P, C], F32, tag="wkv")
                nc.sync.dma_start(wk, kvw_in[c * P:(c + 1) * P, 0:C])
                kvw_k.append(wk)

            # KT [k_col_tile, kv]: kvw_k stationary, ctx_T moving
            for colt in range(C // P):
                for kvch in range(NKV // 512):
                    ps = psp.tile([P, 512], F32)
                    for c in range(C // P):
                        nc.tensor.matmul(
                            ps,
                            _mm_ap(kvw_k[c][:, colt * P:(colt + 1) * P]),
                            _mm_ap(ctx_t[:, c, kvch * 512:(kvch + 1) * 512]),
                            start=(c == 0), stop=(c == C // P - 1))
                    st = ev.tile([P, 512], F32, tag="kst")
                    nc.scalar.copy(st, ps)
                    nc.sync.dma_start(
                        kt_dram[colt, :, kvch * 512:(kvch + 1) * 512], st)

            # V [kv_tile, hd]: ctx_T stationary, kvw_v moving.
            # kvw_v tiles reuse the wkv slots (WAR deps handled by Tile).
            kvw_v = []
            for c in range(C // P):
                wv = wkv.tile([P, C], F32, tag="wkv")
                nc.sync.dma_start(wv, kvw_in[c * P:(c + 1) * P, C:2 * C])
                kvw_v.append(wv)
            for vt in range(NKV // P):
                for hdch in range(C // 512):
                    ps = psp.tile([P, 512], F32)
                    for c in range(C // P):
                        nc.tensor.matmul(
                            ps,
                            _mm_ap(ctx_t[:, c, vt * P:(vt + 1) * P]),
                            _mm_ap(kvw_v[c][:, hdch * 512:(hdch + 1) * 512]),
                            start=(c == 0), stop=(c == C // P - 1))
                    st = ev.tile([P, 512], F32, tag="vst")
                    nc.vector.tensor_copy(st, ps)
                    nc.sync.dma_start(
                        v_dram[vt, :, hdch * 512:(hdch + 1) * 512], st)

        # ---------------- Phase E: attention per head pair ----------------
        if PHASES < 3:
            return
        o_pool = ctx.enter_context(tc.tile_pool(name="o_pool", bufs=1))
        o_sb = o_pool.tile([P, C // P, NQ], F32)        # O^T [hd, q]: 32KB/p

        NKT = NKV // P  # 16 k tiles
        with tc.tile_pool(name="kv_e", bufs=2) as kv_e, \
             tc.tile_pool(name="epool", bufs=2 * NKT + 2) as epool, \
             tc.tile_pool(name="rp", bufs=4) as rp, \
             tc.tile_pool(name="ost", bufs=2) as ostp, \
             tc.tile_pool(name="ps_s", bufs=4, space="PSUM") as ps_s, \
             tc.tile_pool(name="ps_pv", bufs=2, space="PSUM") as ps_pv:
            for hp in range(H // 2):
                ktp = kv_e.tile([P, NKV], F32, tag="ktp")     # 8KB/p
                nc.sync.dma_start(ktp, kt_dram[hp])
                vp = kv_e.tile([P, NKT, 2, D + 1], F32, tag="vp")  # 8.1KB/p
                nc.vector.memset(vp[:, :, :, D:D + 1], 1.0)
                for hh in range(2):
                    h = 2 * hp + hh
                    nc.sync.dma_start(
                        vp[:, :, hh, 0:D],
                        v_dram[:, :, h * D:(h + 1) * D].transpose([1, 0, 2]))

                for qh in range(NQ // 512):
                    qs = slice(qh * 512, (qh + 1) * 512)
                    e_tiles = [[None] * NKT for _ in range(2)]
                    for kt in range(NKT):
                        for hh in range(2):
                            ps = ps_s.tile([P, 512], F32)
                            nc.tensor.matmul(
                                ps,
                                _mm_ap(ktp[hh * D:(hh + 1) * D,
                                           kt * P:(kt + 1) * P]),
                                _mm_ap(qt_sb[hh * D:(hh + 1) * D, hp, qs]),
                                start=True, stop=True)
                            et = epool.tile([P, 512], F32, tag="e")
                            nc.scalar.activation(
                                et, ps, mybir.ActivationFunctionType.Exp,
                                scale=SCALE)
                            e_tiles[hh][kt] = et
                    for hh in range(2):
                        h = 2 * hp + hh
                        pso = ps_pv.tile([P, 512], F32)
                        for kt in range(NKT):
                            nc.tensor.matmul(
                                pso[0:D + 1, :],
                                _mm_ap(vp[:, kt, hh, :]),
                                _mm_ap(e_tiles[hh][kt]),
                                start=(kt == 0), stop=(kt == NKT - 1))
                        # reciprocal of the softmax denominator (row 64),
                        # broadcast to 64 partitions via a DRAM bounce
                        # (SBUF-source partition-step-0 DMA is illegal).
                        rrow = rp.tile([P, 512], F32, tag="rrow")
                        nc.vector.reciprocal(rrow[D:D + 1, :], pso[D:D + 1, :])
                        rd = rdram.tile([1, 512], F32, tag="rd")
                        nc.sync.dma_start(rd, rrow[D:D + 1, :])
                        rbc = rp.tile([D, 512], F32, tag="rbc")
                        nc.sync.dma_start(
                            rbc, rd.partition_broadcast(D))
                        if hh == 0:
                            nc.vector.tensor_mul(
                                o_sb[0:D, hp, qs], pso[0:D, :], rbc)
                        else:
                            ost = ostp.tile([D, 512], F32, tag="ost")
                            nc.vector.tensor_mul(ost, pso[0:D, :], rbc)
                            nc.sync.dma_start(o_sb[D:2 * D, hp, qs], ost)

        # ---------------- Phase F: final projection ----------------
        if PHASES < 4:
            return
        with tc.tile_pool(name="wp", bufs=9) as wpp, \
             tc.tile_pool(name="fin", bufs=3) as finp, \
             tc.tile_pool(name="psp", bufs=4, space="PSUM") as psp:
            bias_bc = wpp.tile([P, C], F32, tag="bias")
            pb2 = pb_in.unsqueeze(0)  # [1, C]
            nc.sync.dma_start(bias_bc, pb2.partition_broadcast(P))
            pw_sb = []
            for hc in range(C // P):
                w = wpp.tile([P, C], F32, tag="pw")
                nc.sync.dma_start(w, pw_in[hc * P:(hc + 1) * P, :])
                pw_sb.append(w)
            for qt in range(NQ // P):
                for cch in range(C // 512):
                    ps = psp.tile([P, 512], F32)
                    for hc in range(C // P):
                        nc.tensor.matmul(
                            ps,
                            _mm_ap(o_sb[:, hc, qt * P:(qt + 1) * P]),
                            _mm_ap(pw_sb[hc][:, cch * 512:(cch + 1) * 512]),
                            start=(hc == 0), stop=(hc == C // P - 1))
                    ft = finp.tile([P, 512], F32, tag="fin")
                    nc.vector.tensor_add(ft, ps, bias_bc[:, cch * 512:(cch + 1) * 512])
                    nc.sync.dma_start(
                        out_d[qt * P:(qt + 1) * P, cch * 512:(cch + 1) * 512], ft)


_CACHED_NC = None


def _get_nc():
    global _CACHED_NC
    if _CACHED_NC is None:
        _CACHED_NC = _build_kernel()
    return _CACHED_NC


def kernel(x, context, q_w, kv_w, proj_w, proj_b):
    x = np.ascontiguousarray(np.asarray(x, dtype=np.float32))
    context = np.ascontiguousarray(np.asarray(context, dtype=np.float32))
    q_w = np.ascontiguousarray(np.asarray(q_w, dtype=np.float32))
    kv_w = np.ascontiguousarray(np.asarray(kv_w, dtype=np.float32))
    proj_w = np.ascontiguousarray(np.asarray(proj_w, dtype=np.float32))
    proj_b = np.ascontiguousarray(np.asarray(proj_b, dtype=np.float32))

    nc = _get_nc()
    in_maps = [
        {
            "x": x[i],
            "context": context,
            "q_w": q_w,
            "kv_w": kv_w,
            "proj_w": proj_w,
            "proj_b": proj_b,
        }
        for i in range(B)
    ]
    res = run_bass_kernel_spmd(nc, in_maps, core_ids=list(range(B)))
    out = np.stack([res.results[i]["out"] for i in range(B)], axis=0)
    return out
